# revision 20
# baseline (speedup 1.0000x reference)
"""NLI classifier (embedding -> shared-weight LSTM x2 -> MLP) on 8 trn2 cores.

Strategy (pure data parallel, TRANSPOSED layout):
  - 1024 sequence instances (512 s1 + 512 s2) sharded 128/core: core k owns
    batch rows [64k, 64k+64) of BOTH s1 (stream A) and s2 (stream B).
  - Layout: gates/hidden on the PARTITION dim, batch on the FREE dim.
    * gatesT psum tile [128, 8, 64]: partition = gate row within chunk,
      free = (gate chunk m, batch). Chunk order [i0 i1 g0 g1 f0 f1 o0 o1],
      g pre-scaled by 2 so tanh(x) = 2*sigmoid(2x)-1 needs one DVE fixup.
    * h state [128, 2, 64] bf16 is produced by DVE directly in the rhs
      layout the recurrent matmul needs: NO transposes, NO psum copies.
    * elementwise ops are full 128-partition width with only 128 elems/lane
      (vs [64, 256] = 256/lane half-width in the normal layout).
  - Host precompute: table2[v] = emb[v] @ w_ih.T + (b_ih+b_hh), permuted to
    chunk order and g-scaled, bf16. Host gathers xgT [128, T, 8, 64] per
    stream per core (bias folded -> inject is a single N=512 id128 matmul).
  - Per step per stream: inject (2 matmuls, one per gate-half bank, emitted
    LOOK steps ahead), recur (16 matmuls N=64, lhsT = constant whh tiles),
    sigmoid split [i,g | f,o] reading the two separate psum banks, DVE cell
    update, tanh, h-mult. Two streams pipeline across PE/ACT/DVE with a
    half-slot phase offset (each stream's tanh+h emitted in the other's
    half-slot). The ig/fo halves live in SEPARATE psum banks so sig_ig only
    depends on the ig-bank writers and fires mid-recur-burst.
  - PE p-state: filler matmuls (written into the about-to-be-injected bank,
    wiped by the inject's start=True) keep the tensor engine continuously
    busy so it ramps to 2.4 GHz and the chip clock stays up.
  - MLP head on device from final h tiles; output [3, 64] f32 per core.
"""

import numpy as np
import ml_dtypes

import concourse.bass as bass
import concourse.bacc as bacc
import concourse.mybir as mybir
import concourse.tile as tile
from concourse.bass_utils import run_bass_kernel_spmd

BF16 = ml_dtypes.bfloat16

VOCAB = 50000
E = 128
H = 256
G = 4 * H          # 1024
B = 512
T = 256
N_CORES = 8
PB = B // N_CORES  # 64 rows per core per stream
NM = 8             # gate chunks of 128
CH = 16            # timesteps per xg DMA chunk
NBUF = 2           # psum buffers per (stream, gate-half) tag
LOOK = 1           # inject emission lead: with NBUF=2, inject(t+1)'s WAR
                   # partner is step t-1's sigmoids (already done), so the
                   # inject never stalls at the head of the PE queue
N_FILL = 12        # PE filler matmuls per half-slot: holds the PE p-state
                   # AND overall chip activity up (low PE duty measurably
                   # slows ACT/DVE clocks ~20% too)

FP32 = mybir.dt.float32
BF = mybir.dt.bfloat16
AF = mybir.ActivationFunctionType
ALU = mybir.AluOpType

_CACHE = {}

# new gate order: [i, g, f, o] in chunks of 256 (2 chunks of 128 each)
_PERM = np.concatenate([
    np.arange(0, 256),      # i
    np.arange(512, 768),    # g
    np.arange(256, 512),    # f
    np.arange(768, 1024),   # o
])
_SCALE = np.ones(G, np.float32)
_SCALE[256:512] = 2.0       # g rows (new order) scaled for 2*sig(2x)-1


def _build(n_fill=N_FILL):
    nc = bacc.Bacc("TRN2", target_bir_lowering=False, debug=False,
                   num_devices=N_CORES)

    xg_in = [
        nc.dram_tensor(f"xg{s}", [128, T, NM, PB], BF,
                       kind="ExternalInput").ap()
        for s in range(2)
    ]
    whh_in = nc.dram_tensor("whh", [128, 2, NM, 128], BF,
                            kind="ExternalInput").ap()
    id128_in = nc.dram_tensor("id128", [128, 128], BF, kind="ExternalInput").ap()
    whid_in = nc.dram_tensor("whid", [128, 4, 2, 128], BF,
                             kind="ExternalInput").ap()
    bhid_in = nc.dram_tensor("bhid", [128, 2], FP32, kind="ExternalInput").ap()
    wout_in = nc.dram_tensor("wout", [128, 2, 3], BF, kind="ExternalInput").ap()
    bout_in = nc.dram_tensor("bout", [3, 1], FP32, kind="ExternalInput").ap()
    out_dram = nc.dram_tensor("out", [3, PB], FP32, kind="ExternalOutput").ap()

    with tile.TileContext(nc) as tc:
        with (
            tc.tile_pool(name="const", bufs=1) as cpool,
            tc.tile_pool(name="state", bufs=1) as spool,
            tc.tile_pool(name="xg", bufs=2) as xgpool,
            tc.tile_pool(name="gpsum", bufs=NBUF, space="PSUM") as gpsum,
        ):
            # ---- constants ----
            whh = cpool.tile([128, 2, NM, 128], BF, tag="whh")
            nc.sync.dma_start(out=whh[:], in_=whh_in[:, :, :, :])
            id128 = cpool.tile([128, 128], BF, tag="id128")
            nc.sync.dma_start(out=id128[:], in_=id128_in[:, :])
            whid = cpool.tile([128, 4, 2, 128], BF, tag="whid")
            nc.sync.dma_start(out=whid[:], in_=whid_in[:, :, :, :])
            bhid = cpool.tile([128, 2], FP32, tag="bhid")
            nc.sync.dma_start(out=bhid[:], in_=bhid_in[:, :])
            wout = cpool.tile([128, 2, 3], BF, tag="wout")
            nc.sync.dma_start(out=wout[:], in_=wout_in[:, :, :])
            bout = cpool.tile([3, 1], FP32, tag="bout")
            nc.sync.dma_start(out=bout[:], in_=bout_in[:, :])

            # ---- per-stream state ----
            c_st = [spool.tile([128, 2 * PB], FP32, tag=f"c{s}", name=f"c{s}")
                    for s in range(2)]
            h_st = [spool.tile([128, 2 * PB], BF, tag=f"h{s}", name=f"h{s}")
                    for s in range(2)]
            sig_ig = [spool.tile([128, 4 * PB], BF, tag=f"sig{s}", name=f"sg{s}")
                      for s in range(2)]
            sig_fo = [spool.tile([128, 4 * PB], BF, tag=f"sf{s}", name=f"sf{s}")
                      for s in range(2)]
            g2 = [spool.tile([128, 2 * PB], BF, tag=f"g2{s}", name=f"g2{s}")
                  for s in range(2)]
            u_t = [spool.tile([128, 2 * PB], BF, tag=f"u{s}", name=f"u{s}")
                   for s in range(2)]
            tc_t = [spool.tile([128, 2 * PB], BF, tag=f"tc{s}", name=f"tc{s}")
                    for s in range(2)]
            xg_tiles = {}   # (s, chunk) -> tile
            ps_tiles = {}   # (s, t) -> psum tile

            def dma_chunk(s, chunk):
                t0 = chunk * CH
                xt = xgpool.tile([128, CH, NM, PB], BF, tag=f"xg{s}",
                                 name=f"xgt{s}")
                nc.sync.dma_start(out=xt[:], in_=xg_in[s][:, t0:t0 + CH, :, :])
                xg_tiles[(s, chunk)] = xt

            def inject(s, t, n_fill=0):
                # ig and fo halves in SEPARATE psum banks: each bank is one
                # accumulation context, so sig_ig's dependency is only the
                # ig-bank writers (inject_ig + the first 8 recur matmuls)
                # and it fires ~300ns before the fo-half finishes. Fillers
                # (p-state/chip-clock hold) write garbage into the ig bank
                # first; the real inject's start=True resets it.
                pig = gpsum.tile([128, 4, PB], FP32, tag=f"pig{s}",
                                 name=f"pig{s}")
                pfo = gpsum.tile([128, 4, PB], FP32, tag=f"pfo{s}",
                                 name=f"pfo{s}")
                ps_tiles[(s, t)] = (pig, pfo)
                xt = xg_tiles[(s, t // CH)]
                for _ in range(n_fill):
                    nc.tensor.matmul(pig[:, 0, :], lhsT=id128[:],
                                     rhs=h_st[s][:, 0:PB],
                                     start=True, stop=True,
                                     skip_group_check=True)
                nc.tensor.matmul(pig[:, :, :], lhsT=id128[:],
                                 rhs=xt[:, t % CH, 0:4, :],
                                 start=True, stop=(t == 0),
                                 skip_group_check=True)
                nc.tensor.matmul(pfo[:, :, :], lhsT=id128[:],
                                 rhs=xt[:, t % CH, 4:8, :],
                                 start=True, stop=(t == 0),
                                 skip_group_check=True)

            def recur(s, t):
                pig, pfo = ps_tiles[(s, t)]
                for m in range(NM):
                    out = pig[:, m, :] if m < 4 else pfo[:, m - 4, :]
                    for k in range(2):
                        nc.tensor.matmul(out,
                                         lhsT=whh[:, k, m, :],
                                         rhs=h_st[s][:, k * PB:(k + 1) * PB],
                                         start=False, stop=(k == 1),
                                         skip_group_check=True)

            def act_sigs(s, t):
                pig, pfo = ps_tiles[(s, t)]
                nc.scalar.activation(sig_ig[s][:], pig[:, :, :], AF.Sigmoid)
                nc.scalar.activation(sig_fo[s][:], pfo[:, :, :], AF.Sigmoid)

            def dve_cell(s, t):
                # g = 2*sig(2x)-1 ; u = i*g
                nc.vector.tensor_scalar(g2[s][:], sig_ig[s][:, 2 * PB:4 * PB],
                                        2.0, -1.0, op0=ALU.mult, op1=ALU.add)
                nc.vector.tensor_tensor(u_t[s][:], sig_ig[s][:, 0:2 * PB],
                                        g2[s][:], op=ALU.mult)
                if t == 0:
                    nc.vector.tensor_copy(c_st[s][:], u_t[s][:])
                else:
                    nc.vector.tensor_tensor(c_st[s][:], sig_fo[s][:, 0:2 * PB],
                                            c_st[s][:], op=ALU.mult)
                    nc.vector.tensor_tensor(c_st[s][:], c_st[s][:],
                                            u_t[s][:], op=ALU.add)

            def act_tanh(s):
                nc.scalar.activation(tc_t[s][:], c_st[s][:], AF.Tanh)

            def dve_h(s):
                nc.vector.tensor_tensor(h_st[s][:], sig_fo[s][:, 2 * PB:4 * PB],
                                        tc_t[s][:], op=ALU.mult)

            # ---- prologue ----
            for s in range(2):
                dma_chunk(s, 0)
            for t in range(LOOK):
                for s in range(2):
                    inject(s, t)

            # ---- main loop ----
            # Cyclic emission for phi = P/2 two-stream overlap: each
            # stream's tanh+h are emitted in the OTHER stream's half-slot so
            # they ride during its recur burst instead of blocking behind
            # its sigmoids in the in-order ACT queue.
            for t in range(T):
                if t % CH == 0 and t + CH < T:
                    for s in range(2):
                        dma_chunk(s, t // CH + 1)
                for s in range(2):
                    if t > 0:
                        recur(s, t)
                    if t + LOOK < T:
                        inject(s, t + LOOK, n_fill if t > 0 else 0)
                    if s == 0:
                        if t > 0:
                            act_tanh(1)      # B's tail of step t-1
                            dve_h(1)
                    else:
                        act_tanh(0)          # A's tail of step t
                        dve_h(0)
                    act_sigs(s, t)
                    dve_cell(s, t)
            act_tanh(1)                      # B's tail of step T-1
            dve_h(1)

            # ---- MLP head ----
            catT = [h_st[0][:, 0:PB], h_st[0][:, PB:2 * PB],
                    h_st[1][:, 0:PB], h_st[1][:, PB:2 * PB]]
            hp = gpsum.tile([128, 4, PB], FP32, tag="pig0", name="hp")
            for m in range(2):
                for k4 in range(4):
                    nc.tensor.matmul(hp[:, m, :],
                                     lhsT=whid[:, k4, m, :], rhs=catT[k4],
                                     start=(k4 == 0), stop=(k4 == 3),
                                     skip_group_check=True)
            hidT = spool.tile([128, 2, PB], BF, tag="hidT")
            for m in range(2):
                nc.scalar.activation(hidT[:, m, :], hp[:, m, :], AF.Relu,
                                     bias=bhid[:, m:m + 1])
            lp = gpsum.tile([128, 4, PB], FP32, tag="pfo0", name="lp")
            for m in range(2):
                nc.tensor.matmul(lp[0:3, 0, :], lhsT=wout[:, m, :],
                                 rhs=hidT[:, m, :],
                                 start=(m == 0), stop=(m == 1),
                                 skip_group_check=True)
            logits = spool.tile([3, PB], FP32, tag="logits")
            nc.scalar.activation(logits[:], lp[0:3, 0, :], AF.Identity,
                                 bias=bout[:, 0:1])
            nc.sync.dma_start(out=out_dram[:, :], in_=logits[:])

    nc.compile()
    return nc


LAST_RESULT = None


def kernel(s1, s2, emb, w_ih, w_hh, b_ih, b_hh, w_hid, b_hid, w_out, b_out,
           _trace=False):
    global LAST_RESULT
    s1 = np.asarray(s1)
    s2 = np.asarray(s2)
    emb = np.asarray(emb, np.float32)
    w_ih = np.asarray(w_ih, np.float32)
    w_hh = np.asarray(w_hh, np.float32)
    b_ih = np.asarray(b_ih, np.float32)
    b_hh = np.asarray(b_hh, np.float32)
    w_hid = np.asarray(w_hid, np.float32)
    b_hid = np.asarray(b_hid, np.float32)
    w_out = np.asarray(w_out, np.float32)
    b_out = np.asarray(b_out, np.float32)

    # host precompute: projected+biased gate table in chunk order, g scaled
    Wg = w_ih[_PERM] * _SCALE[:, None]
    bias = (b_ih + b_hh)[_PERM] * _SCALE
    table2 = (emb @ Wg.T + bias).astype(BF16)           # [V, G] new order
    # whh [128(p), 2(k), 8(m), 128(q)]
    whh_new = (w_hh.T[:, _PERM] * _SCALE[None, :])      # [H, G]
    whh_dev = np.ascontiguousarray(
        whh_new.reshape(2, 128, NM, 128).transpose(1, 0, 2, 3)).astype(BF16)
    whid_dev = np.ascontiguousarray(
        w_hid.T.reshape(4, 128, 2, 128).transpose(1, 0, 2, 3)).astype(BF16)
    bhid_dev = np.ascontiguousarray(
        b_hid.reshape(2, 128).T).astype(np.float32)
    wout_dev = np.ascontiguousarray(
        w_out.T.reshape(2, 128, 3).transpose(1, 0, 2)).astype(BF16)
    bout_dev = b_out.reshape(3, 1).astype(np.float32)
    id128 = np.eye(128, dtype=BF16)

    key = "v2"
    if key not in _CACHE:
        _CACHE[key] = _build()
    nc = _CACHE[key]

    def gather(tok):
        # tok [PB, T] -> xgT [128, T, NM, PB]
        g = table2[tok]                                  # [PB, T, G]
        g = g.reshape(PB, T, NM, 128)
        return np.ascontiguousarray(g.transpose(3, 1, 2, 0))

    in_maps = []
    for k in range(N_CORES):
        sl = slice(k * PB, (k + 1) * PB)
        in_maps.append({
            "xg0": gather(s1[sl]),
            "xg1": gather(s2[sl]),
            "whh": whh_dev,
            "id128": id128,
            "whid": whid_dev,
            "bhid": bhid_dev,
            "wout": wout_dev,
            "bout": bout_dev,
        })

    res = run_bass_kernel_spmd(nc, in_maps, list(range(N_CORES)), trace=_trace)
    LAST_RESULT = res
    out = np.empty((B, 3), np.float32)
    for k in range(N_CORES):
        out[k * PB:(k + 1) * PB] = res.results[k]["out"].T
    return out
